# revision 4
# baseline (speedup 1.0000x reference)
"""Trainium2 Bass kernel for the minGRU encoder — hybrid fp8/bf16 variant.

Strategy
--------
- Data-parallel over batch: 16 batches -> 8 cores x 2 batches.
- Per layer, the GATE matmul (z-path) runs in fp8e4m3 with
  perf_mode=DoubleRow (contraction 256/instruction, 2x bf16 rate); gate
  errors are squashed by the sigmoid, so fp8 noise there is harmless.
- The CANDIDATE matmul runs bf16 x bf16: its output feeds the scan
  directly, so it needs bf16-class accuracy (full fp8 measured 7e-2 rel
  err vs the 2e-2 budget; this hybrid measures ~6e-3).
- Dual hidden-state storage, both holding S*h (S=16): h8 (fp8,
  DoubleRow-interleaved [128, Tg, 2] per 256-feature superblock) feeds the
  gate; hb (bf16 [128, Tg] per 128-feature block) feeds the candidate and
  the post-projection. The DVE scan writes hb; a DVE tensor_copy casts
  hb -> h8 (gpsimd and scalar casts both measured slower).
- Gate weights (all 6 layers, fp8 sigma=64-scaled, 6.3MB) are persistent
  in SBUF; candidate weights stream per layer in bf16 (2.1MB/layer).
- Scan inputs a, b stay f32 (bf16 a/b alone measured 1.3e-2 rel err).
- Time axis in G=2 groups of 2048 to fit SBUF; per-layer f32 carries
  bridge the groups; batches reset the state (h0 = 0).
- The post-projection (bf16) is fused into layer 6's chunk loop, reading
  hb; pre-projection is bf16, writing both hb and h8.

Measured (8 trn2 cores, REP-differential): 2.46 ms HW exec, rel err
6.6e-3 vs the f32 reference (2e-2 budget); the f32r baseline was 3.57 ms.
Calibration notes from this hardware: DR fp8 MMs sustain ~219 ns per
[256x128x512] instruction (LDWEIGHTS hidden); bf16 MMs sustain ~272 ns per
[128x128x512] (FWL LDWEIGHTS partially exposed), which makes the measured
time match the 32 DR + 64 bf16 per-chunk PE floor almost exactly.
A bf16->fp8 strided tensor_copy is slow on every engine (~1.5-2 us per
[128,512] tile); only the DVE scan writes fp8-strided cheaply.
"""

import numpy as np
import ml_dtypes

import concourse.bass as bass
import concourse.mybir as mybir
import concourse.tile as tile

# --- walrus single-wait workaround (same as baseline kernel.py) ---
from concourse.vector_clock import ScopedClock

_MAX_WAITS = 1
_noop_ctr = [0]


def _split_waits_in_block(bb):
    new_list = []
    for inst in bb.instructions:
        si = getattr(inst, "sync_info", None)
        if si is not None and si.on_wait and len(si.on_wait) > _MAX_WAITS:
            waits = list(si.on_wait)
            keep = waits[-_MAX_WAITS:]
            extra = waits[:-_MAX_WAITS]
            for i in range(0, len(extra), _MAX_WAITS):
                _noop_ctr[0] += 1
                nop = mybir.InstNoOp(
                    name=f"I-waitsplit-{_noop_ctr[0]}",
                    engine=inst.engine,
                    bass_nofuse=True,
                    sync_info=mybir.SyncInfo(
                        on_wait=extra[i : i + _MAX_WAITS], on_update=[]
                    ),
                )
                new_list.append(nop)
            inst.sync_info = mybir.SyncInfo(on_wait=keep, on_update=si.on_update)
        new_list.append(inst)
    bb.instructions[:] = new_list


def _patched_drain_and_barrier(self, tick_clock, wait_clock):
    nc = self.nc
    drain_inst = nc.sync.drain()
    wait_clock.add_sem_waits(
        drain_inst.ins, ScopedClock({None: tick_clock.global_clock})
    )
    for bb in nc.main_func.blocks:
        _split_waits_in_block(bb)
    nc.all_engine_barrier()
    assert self.sems is not None
    popped = nc._tile_sem_poison_stack.pop()
    assert popped is self._sem_poison
    nc.clear_and_free_semaphores(list(self.sems.allocated().values()))
    nc.all_engine_barrier()


tile.TileContext._drain_and_barrier = _patched_drain_and_barrier
# ------------------------------------------------------------------

f32 = mybir.dt.float32
f32r = mybir.dt.float32r
f8 = mybir.dt.float8e4
bf16 = mybir.dt.bfloat16
u8 = mybir.dt.uint8
AF = mybir.ActivationFunctionType
ALU = mybir.AluOpType
DR = mybir.MatmulPerfMode.DoubleRow

N_CORES = 8
C_IN = 80
C_OUT = 194
D = 1024
NJ = D // 128     # 8 feature blocks
NSB = D // 256    # 4 superblocks (DoubleRow pairs)

SIGMA = 64.0      # gate weight scale
S_H = 16.0        # hidden-state scale


def build_program(L=6, T=4096, G=2, S=512, BS=2, REP=1, cast_eng="gpsimd",
                  zpath="dr", order="jint", ab16=True, layout="il"):
    Tg = T // G
    NCH = Tg // S
    nc = bass.Bass()

    x_d = nc.declare_dram_parameter("x", [BS, C_IN, T], bf16, isOutput=False)
    wpre_d = nc.declare_dram_parameter("wpre", [C_IN, D], bf16, isOutput=False)
    bpre_d = nc.declare_dram_parameter("bpre", [128, NJ], f32, isOutput=False)
    wz8_d = nc.declare_dram_parameter(
        "wz8", [L, NJ, 128, NSB, 2, 128], u8, isOutput=False
    )
    wcb_d = nc.declare_dram_parameter("wcb", [L, NJ, 128, D], bf16, isOutput=False)
    wzb_d = nc.declare_dram_parameter("wzb", [L, NJ, 128, D], bf16, isOutput=False)
    bl_d = nc.declare_dram_parameter("bl", [128, L * NJ], f32, isOutput=False)
    wpost_d = nc.declare_dram_parameter(
        "wpost", [128, NJ * C_OUT], bf16, isOutput=False
    )
    bpost_d = nc.declare_dram_parameter("bpost", [128, 2], f32, isOutput=False)
    out_d = nc.declare_dram_parameter("out", [BS, C_OUT, T], f32, isOutput=True)

    with tile.TileContext(nc) as tc:
        with (
            tc.tile_pool(name="const", bufs=1) as cpool,
            tc.tile_pool(name="h", bufs=1) as hpool,
            tc.tile_pool(name="wc", bufs=1) as wpool,
            tc.tile_pool(name="scr", bufs=1) as spool,
            tc.tile_pool(name="ps", bufs=1, space="PSUM") as pspool,
        ):
            # ---- constants / gate weights loaded once ----
            wpre_sb = cpool.tile([C_IN, D], bf16, tag="wpre")
            nc.sync.dma_start(wpre_sb[:], wpre_d[:])
            bpre_sb = cpool.tile([128, NJ], f32, tag="bpre")
            nc.sync.dma_start(bpre_sb[:], bpre_d[:])
            wpost_sb = cpool.tile([128, NJ * C_OUT], bf16, tag="wpost")
            nc.sync.dma_start(wpost_sb[:], wpost_d[:])
            bpost_sb = cpool.tile([128, 2], f32, tag="bpost")
            nc.sync.dma_start(bpost_sb[:], bpost_d[:])

            wz8 = []
            if zpath == "dr":
                for i in range(L):
                    row = []
                    for j in range(NJ):
                        wt = cpool.tile([128, NSB, 2, 128], f8,
                                        tag=f"w{i}_{j}", name=f"w{i}_{j}")
                        nc.sync.dma_start(wt[:].bitcast(u8), wz8_d[i, j])
                        row.append(wt)
                    wz8.append(row)
            bz_sb = cpool.tile([128, L * NJ], f32, tag="bz")
            nc.sync.dma_start(bz_sb[:], bl_d[:])
            nbz_sb = cpool.tile([128, L * NJ], f32, tag="nbz")
            nc.scalar.mul(nbz_sb[:], bz_sb[:], -1.0)
            carry_sb = cpool.tile([128, L * NJ], f32, tag="carry")
            zeros_sb = cpool.tile([128, 512], bf16, tag="zeros")
            nc.vector.memset(zeros_sb[:], 0.0)

            # ---- persistent hidden state (per group) ----
            if zpath == "dr" and layout == "il":
                h8 = [hpool.tile([128, Tg, 2], f8, tag=f"h8_{sb}",
                                 name=f"h8_{sb}") for sb in range(NSB)]

                def h8_rhs(sb, t0, S_):   # [128, 2, S] ifmap (interleaved)
                    return h8[sb][:, t0 : t0 + S_, :].transpose([0, 2, 1])

                def h8_out(j, t0, S_):    # [128, S] strided write target
                    return h8[j // 2][:, t0 : t0 + S_, j % 2]
            elif zpath == "dr":  # q-halves: contiguous writes, strided ifmap
                h8 = [hpool.tile([128, 2, Tg], f8, tag=f"h8_{sb}",
                                 name=f"h8_{sb}") for sb in range(NSB)]

                def h8_rhs(sb, t0, S_):
                    return h8[sb][:, :, t0 : t0 + S_]

                def h8_out(j, t0, S_):
                    return h8[j // 2][:, j % 2, t0 : t0 + S_]
            hb = [hpool.tile([128, Tg], bf16, tag=f"hb_{j}", name=f"hb_{j}")
                  for j in range(NJ)]

            cast = (nc.gpsimd.tensor_copy if cast_eng == "gpsimd"
                    else (nc.vector.tensor_copy if cast_eng == "vector"
                          else nc.scalar.copy))

            for _rep in range(REP):
              for b in range(BS):
                for g in range(G):
                    tg0 = g * Tg
                    # ---- input slab ----
                    x_sb = spool.tile([C_IN, Tg], bf16, tag="x", name="x_sb")
                    nc.sync.dma_start(x_sb[:], x_d[b][:, tg0 : tg0 + Tg])

                    # ---- pre-projection -> hb, h8 (= S*(x W_pre + b_pre)) --
                    for c in range(NCH):
                        t0 = c * S
                        for j in range(NJ):
                            ps = pspool.tile(
                                [128, S], f32, tag="pz",
                                bufs=(3 if order == "jint" else 5),
                                name="ps_pre")
                            nc.tensor.matmul(
                                ps[:],
                                wpre_sb[:, j * 128 : (j + 1) * 128],
                                x_sb[:, t0 : t0 + S],
                                start=True, stop=True,
                            )
                            nc.scalar.activation(
                                hb[j][:, t0 : t0 + S], ps[:], AF.Identity,
                                bias=bpre_sb[:, j : j + 1], scale=S_H,
                            )
                            if zpath == "dr":
                                if cast_eng == "scan":
                                    nc.vector.tensor_tensor_scan(
                                        h8_out(j, t0, S), zeros_sb[:, :S],
                                        hb[j][:, t0 : t0 + S], 0.0,
                                        op0=ALU.mult, op1=ALU.add,
                                    )
                                else:
                                    cast(h8_out(j, t0, S),
                                         hb[j][:, t0 : t0 + S])

                    # ---- recurrent layers ----
                    for i in range(L):
                        last = i == L - 1
                        wcb = []
                        for j in range(NJ):
                            wt = wpool.tile([128, D], bf16, tag=f"wc{j}",
                                            bufs=2, name=f"wc{j}")
                            nc.sync.dma_start(wt[:], wcb_d[i, j])
                            wcb.append(wt)
                        if zpath == "bf16":
                            wzb = []
                            for j in range(NJ):
                                wt = wpool.tile([128, D], bf16, tag=f"wzb{j}",
                                                bufs=2, name=f"wzb{j}")
                                nc.sync.dma_start(wt[:], wzb_d[i, j])
                                wzb.append(wt)
                        for c in range(NCH):
                            t0 = c * S
                            ats, bts = [], []
                            pszs, pscs = [], []

                            def emit_z(j):
                                psz = pspool.tile(
                                    [128, S], f32, tag="pz",
                                    bufs=(3 if order == "jint" else 5),
                                    name="psz")
                                if zpath == "dr":
                                    for sb in range(NSB):
                                        nc.tensor.matmul(
                                            psz[:], wz8[i][j][:, sb],
                                            h8_rhs(sb, t0, S),
                                            start=(sb == 0),
                                            stop=(sb == NSB - 1),
                                            perf_mode=DR,
                                        )
                                else:
                                    for kb in range(NJ):
                                        nc.tensor.matmul(
                                            psz[:],
                                            wzb[j][:, kb * 128 : (kb + 1) * 128],
                                            hb[kb][:, t0 : t0 + S],
                                            start=(kb == 0),
                                            stop=(kb == NJ - 1),
                                        )
                                return psz

                            def emit_c(j):
                                psc = pspool.tile([128, S], f32, tag="pc",
                                                  bufs=3, name="psc")
                                for kb in range(NJ):
                                    nc.tensor.matmul(
                                        psc[:],
                                        wcb[j][:, kb * 128 : (kb + 1) * 128],
                                        hb[kb][:, t0 : t0 + S],
                                        start=(kb == 0), stop=(kb == NJ - 1),
                                    )
                                return psc

                            if order == "split":
                                pszs = [emit_z(j) for j in range(NJ)]
                                pscs = [emit_c(j) for j in range(NJ)]
                            for j in range(NJ):
                                if order == "split":
                                    psz, psc = pszs[j], pscs[j]
                                else:
                                    psz = emit_z(j)
                                    psc = emit_c(j)
                                dab = bf16 if ab16 else f32
                                z_t = spool.tile([128, S], f32, tag="z",
                                                 bufs=4, name="z_t")
                                a_t = spool.tile([128, S], dab, tag="a",
                                                 bufs=10, name="a_t")
                                b_t = spool.tile([128, S], dab, tag="bb",
                                                 bufs=10, name="b_t")
                                zsc = (1.0 / (SIGMA * S_H) if zpath == "dr"
                                       else 1.0 / S_H)
                                nc.scalar.activation(
                                    z_t[:], psz[:], AF.Sigmoid,
                                    bias=bz_sb[:, i * NJ + j : i * NJ + j + 1],
                                    scale=zsc,
                                )
                                nc.scalar.activation(
                                    a_t[:], psz[:], AF.Sigmoid,
                                    bias=nbz_sb[:, i * NJ + j : i * NJ + j + 1],
                                    scale=-zsc,
                                )
                                # b = z * psc (psc is already S*ch)
                                nc.vector.scalar_tensor_tensor(
                                    b_t[:], psc[:], 1.0, z_t[:],
                                    op0=ALU.mult, op1=ALU.mult,
                                )
                                ats.append(a_t)
                                bts.append(b_t)
                            for j in range(NJ):
                                if c == 0 and g == 0:
                                    init = 0.0
                                elif c == 0:
                                    init = carry_sb[:, i * NJ + j :
                                                    i * NJ + j + 1]
                                else:
                                    init = hb[j][:, t0 - 1 : t0]
                                nc.vector.tensor_tensor_scan(
                                    hb[j][:, t0 : t0 + S], ats[j][:],
                                    bts[j][:], init,
                                    op0=ALU.mult, op1=ALU.add,
                                )
                                if not last and zpath == "dr":
                                    if cast_eng == "scan":
                                        nc.vector.tensor_tensor_scan(
                                            h8_out(j, t0, S), ats[j][:],
                                            bts[j][:], init,
                                            op0=ALU.mult, op1=ALU.add,
                                        )
                                    else:
                                        cast(h8_out(j, t0, S),
                                             hb[j][:, t0 : t0 + S])
                            if last:
                                # ---- fused post-projection ----
                                for p, (p0, pw) in enumerate(
                                        ((0, 128), (128, C_OUT - 128))):
                                    if order == "jint":
                                        ps_o = pspool.tile([128, S], f32,
                                                           tag="po", bufs=2,
                                                           name="ps_o")
                                    else:
                                        ps_o = pspool.tile([128, S], f32,
                                                           tag="pz", bufs=5,
                                                           name="ps_o")
                                    for kb in range(NJ):
                                        nc.tensor.matmul(
                                            ps_o[:pw, :],
                                            wpost_sb[:, kb * C_OUT + p0 :
                                                     kb * C_OUT + p0 + pw],
                                            hb[kb][:, t0 : t0 + S],
                                            start=(kb == 0),
                                            stop=(kb == NJ - 1),
                                        )
                                    o_t = spool.tile([128, S], f32, tag="o",
                                                     bufs=4, name="o_t")
                                    nc.scalar.activation(
                                        o_t[:pw, :], ps_o[:pw, :], AF.Identity,
                                        bias=bpost_sb[:pw, p : p + 1],
                                        scale=1.0 / S_H,
                                    )
                                    nc.sync.dma_start(
                                        out_d[b][p0 : p0 + pw,
                                                 tg0 + t0 : tg0 + t0 + S],
                                        o_t[:pw, :],
                                    )
                        if g == 0:
                            for j in range(NJ):
                                nc.vector.tensor_copy(
                                    carry_sb[:, i * NJ + j : i * NJ + j + 1],
                                    hb[j][:, Tg - 1 : Tg],
                                )
    return nc


def pack_inputs(x, w_pre, b_pre, w_layers, b_layers, w_post, b_post, L=6):
    F8NP = ml_dtypes.float8_e4m3
    BF = ml_dtypes.bfloat16
    x = np.ascontiguousarray(np.asarray(x, dtype=np.float32).astype(BF))
    w_pre = np.ascontiguousarray(np.asarray(w_pre, dtype=np.float32).astype(BF))
    bpre = np.ascontiguousarray(
        (S_H * np.asarray(b_pre, dtype=np.float32)).reshape(NJ, 128).T
    )  # bias slot already includes the S factor
    wl = np.asarray(w_layers, dtype=np.float32)
    # gate half: wz8[i, j, p, sb, q, m] = sigma*W[i, sb*256+q*128+p, j*128+m]
    wz = wl[:, :, :D].reshape(L, NSB, 2, 128, NJ, 128)
    wz = wz.transpose(0, 4, 3, 1, 2, 5)  # [L, j, p, sb, q, m]
    wz8 = np.clip(wz * SIGMA, -240.0, 240.0).astype(F8NP).view(np.uint8)
    wz8 = np.ascontiguousarray(wz8)
    # candidate half: wcb[i, j, kp, kb*128+m] = W[i, kb*128+kp, 1024+j*128+m]
    wc = wl[:, :, D:].reshape(L, NJ, 128, NJ, 128)
    wc = wc.transpose(0, 3, 2, 1, 4).reshape(L, NJ, 128, D)
    wcb = np.ascontiguousarray(wc.astype(BF))
    # gate half in the same bf16 streaming layout (for zpath="bf16")
    wzl = wl[:, :, :D].reshape(L, NJ, 128, NJ, 128)
    wzl = wzl.transpose(0, 3, 2, 1, 4).reshape(L, NJ, 128, D)
    wzb = np.ascontiguousarray(wzl.astype(BF))
    blr = np.asarray(b_layers, dtype=np.float32).reshape(L, 2, NJ, 128)
    assert np.abs(blr[:, 1]).max() == 0.0, "candidate layer biases must be zero"
    bl = np.ascontiguousarray(
        blr[:, 0].transpose(2, 0, 1).reshape(128, L * NJ)
    )
    wpost = (
        np.asarray(w_post, dtype=np.float32)
        .reshape(NJ, 128, C_OUT)
        .transpose(1, 0, 2)
        .reshape(128, NJ * C_OUT)
    )
    wpost = np.ascontiguousarray(wpost.astype(BF))
    bpost = np.zeros((128, 2), dtype=np.float32)
    bpost[:, 0] = np.asarray(b_post[:128], dtype=np.float32)
    bpost[: C_OUT - 128, 1] = np.asarray(b_post[128:], dtype=np.float32)
    return x, w_pre, bpre, wz8, wcb, wzb, bl, wpost, bpost


_program_cache = {}


def _get_program(key):
    if key not in _program_cache:
        L, T, G, S, BS, REP, cast_eng, zpath, order, ab16, layout = key
        _program_cache[key] = build_program(
            L=L, T=T, G=G, S=S, BS=BS, REP=REP, cast_eng=cast_eng,
            zpath=zpath, order=order, ab16=ab16, layout=layout,
        )
    return _program_cache[key]


def run(inputs, L=6, T=4096, G=2, S=512, REP=1, cast_eng="vector",
        zpath="dr", order="jint", ab16=False, layout="il", trace=False):
    from concourse.bass_utils import run_bass_kernel_spmd

    x, w_pre, bpre, wz8, wcb, wzb, bl, wpost, bpost = pack_inputs(
        inputs["x"], inputs["w_pre"], inputs["b_pre"], inputs["w_layers"],
        inputs["b_layers"], inputs["w_post"], inputs["b_post"], L=L,
    )
    B = x.shape[0]
    BS = B // N_CORES
    nc = _get_program((L, T, G, S, BS, REP, cast_eng, zpath, order, ab16,
                       layout))
    shared = {"wpre": w_pre, "bpre": bpre, "wz8": wz8, "wcb": wcb,
              "wzb": wzb, "bl": bl, "wpost": wpost, "bpost": bpost}
    in_maps = [
        {"x": np.ascontiguousarray(x[c * BS : (c + 1) * BS]), **shared}
        for c in range(N_CORES)
    ]
    res = run_bass_kernel_spmd(nc, in_maps, list(range(N_CORES)), trace=trace)
    out = np.concatenate([res.results[c]["out"] for c in range(N_CORES)], axis=0)
    return out, res


def kernel(**inputs):
    out, _ = run(inputs)
    return out
